# revision 36
# baseline (speedup 1.0000x reference)
"""AGA flash-attention (routed slot attention) TRN2 kernel.

Data-parallel over tokens: 16384 tokens split across 8 NeuronCores, slot
memory (keys/values/reliability) replicated. Per core, per 128-token tile:
  1. PE: router scores r = qT.T @ keysT accumulated with a K=1 outer-product
     bias matmul (bias = ln(reliability + eps) per slot).
  2. ACT: drain PSUM -> SBUF.
  3. DVE: max8 (top-8 values, descending) + find_index8 (their slot indices).
  4. GPSIMD indirect DMA: gather bias at the 8 indices (per-token) and the
     8 value rows (bf16) per token.
  5. ACT: e8 = exp((r8-b8)*SCALE) with accumulated denominator; DVE recip.
  6. PE: out = sum_k diag(e8_k) @ V_rows_k (psum f32), drained with a
     per-token 1/denom scale on ACT.  attn_weights = e8 * (1/denom).
Output is packed [tokens, 1024+8] (values || weights) and split on host.
"""

import sys

for _p in ("/opt/trn_rl_repo",):
    if _p not in sys.path:
        sys.path.append(_p)

import numpy as np
import ml_dtypes

import concourse.bass as bass
import concourse.bacc as bacc
import concourse.mybir as mybir
from concourse.tile import TileContext
from concourse.bass_utils import run_bass_kernel_spmd
from concourse import masks

F32 = mybir.dt.float32
I16 = mybir.dt.int16
BF16 = mybir.dt.bfloat16
U32 = mybir.dt.uint32
AF = mybir.ActivationFunctionType
ALU = mybir.AluOpType

B, S, D, N, H, K = 4, 4096, 128, 4096, 1024, 8
N_CORES = 8
TOKENS = B * S
TPC = TOKENS // N_CORES  # tokens per core
P = 128
SCALE = 1.0 / float(np.sqrt(D))
EPS = 1e-10
OUTW = H + K
W = 1152  # augmented row: 1024 values + bias + pad (2304 B, %256 for dma_gather)


def build(tpc=TPC, bias_pe=0, mac_wide=False, skew_a2=3, skew_b=3, gather_split=8):
    n_tiles = tpc // P
    nc = bacc.Bacc("TRN2", target_bir_lowering=False, debug=False, num_swdge_queues=4)
    qT = nc.dram_tensor("qT", [D, tpc], F32, kind="ExternalInput")
    keysT = nc.dram_tensor("keysT", [D, N], F32, kind="ExternalInput")
    vals = nc.dram_tensor("vals", [N, W], BF16, kind="ExternalInput")
    rel = nc.dram_tensor("rel", [1, N], F32, kind="ExternalInput")
    out = nc.dram_tensor("out", [tpc, OUTW], F32, kind="ExternalOutput")
    bias_d = nc.dram_tensor("bias_in", [1, N], F32, kind="ExternalInput")
    rep16 = nc.dram_tensor("rep16", [16, P], F32, kind="ExternalInput")

    with TileContext(nc) as tc:
        with (
            tc.tile_pool(name="const", bufs=1) as cpool,
            tc.tile_pool(name="scores", bufs=2) as spool,
            tc.tile_pool(name="gather", bufs=4) as gpool,
            tc.tile_pool(name="outp", bufs=3) as opool,
            tc.tile_pool(name="small", bufs=10) as smpool,
            tc.tile_pool(name="diag", bufs=8) as dpool,
            tc.tile_pool(name="ps_s", bufs=2, space="PSUM") as pspool,
            tc.tile_pool(name="ps_o", bufs=2, space="PSUM") as popool,
        ):
            qT_sb = cpool.tile([D, tpc], F32)
            nc.sync.dma_start(out=qT_sb[:], in_=qT.ap())
            keysT_sb = cpool.tile([D, N], F32)
            nc.sync.dma_start(out=keysT_sb[:], in_=keysT.ap())
            rel_sb = cpool.tile([1, N], F32)
            nc.sync.dma_start(out=rel_sb[:], in_=rel.ap())
            bias_sb = cpool.tile([1, N], F32)
            nc.sync.dma_start(out=bias_sb[:], in_=bias_d.ap())
            ones_sb = cpool.tile([1, P], F32)
            nc.vector.memset(ones_sb[:], 1.0)
            ident = cpool.tile([P, P], BF16)
            masks.make_identity(nc, ident[:])
            rep_sb = cpool.tile([16, P], F32)
            nc.sync.dma_start(out=rep_sb[:], in_=rep16.ap())
            bias_rep = cpool.tile([P, N], F32)
            nc.gpsimd.partition_broadcast(
                out_ap=bias_rep[:], in_ap=bias_sb[:], channels=P
            )

            def phase_a(i):
                ssb = spool.tile([P, N], F32)
                # chunks 0,1: bias via PE outer-product + ACT drain
                # chunks 2,3: pure score matmul; bias added by the DVE drain
                # (balances PE vs DVE; single qT LDWEIGHTS for all 8 matmuls)
                cand = smpool.tile([P, 32], F32, tag="cand")
                # process chunks in pairs sized to the PSUM pool (2 slots):
                # scores (one qT LDW per pair), then bias matmuls (PE) or
                # bias-added DVE drains, then ACT drains + per-chunk max8.
                for pair in range(2):
                    chunks = (2 * pair, 2 * pair + 1)
                    pss = {}
                    for c in chunks:
                        ps = pspool.tile([P, 1024], F32)
                        pss[c] = ps
                        last = c >= bias_pe
                        for h2 in range(2):
                            sl = slice(c * 1024 + h2 * 512, c * 1024 + (h2 + 1) * 512)
                            nc.tensor.matmul(
                                out=ps[:, h2 * 512 : (h2 + 1) * 512],
                                lhsT=qT_sb[:, i * P : (i + 1) * P],
                                rhs=keysT_sb[:, sl],
                                start=True,
                                stop=last,
                            )
                    for c in chunks:
                        if c < bias_pe:
                            ps = pss[c]
                            for h2 in range(2):
                                sl = slice(
                                    c * 1024 + h2 * 512, c * 1024 + (h2 + 1) * 512
                                )
                                nc.tensor.matmul(
                                    out=ps[:, h2 * 512 : (h2 + 1) * 512],
                                    lhsT=ones_sb[:],
                                    rhs=bias_sb[:, sl],
                                    start=False,
                                    stop=True,
                                )
                    for c in chunks:
                        ps = pss[c]
                        if c < bias_pe:
                            nc.scalar.activation(
                                out=ssb[:, c * 1024 : (c + 1) * 1024],
                                in_=ps[:],
                                func=AF.Copy,
                            )
                        else:
                            nc.vector.scalar_tensor_tensor(
                                out=ssb[:, c * 1024 : (c + 1) * 1024],
                                in0=ps[:],
                                scalar=1.0,
                                in1=bias_rep[:, c * 1024 : (c + 1) * 1024],
                                op0=ALU.mult,
                                op1=ALU.add,
                            )
                        nc.vector.max(
                            out=cand[:, c * 8 : (c + 1) * 8],
                            in_=ssb[:, c * 1024 : (c + 1) * 1024],
                        )

                r8 = smpool.tile([P, K], F32, tag="r8")
                idx = smpool.tile([P, K], U32, tag="idx")
                nc.vector.max(out=r8[:], in_=cand[:])
                nc.vector.max_index(out=idx[:], in_max=r8[:], in_values=ssb[:])

                # build dma_gather's wrapped int16 index layout on-chip:
                # wrapped[16g+q, 8j+a] = idx[16a+q, j] for all groups g.
                # 1) cast to f32; 2) 8 partition-shift DMAs build [16, 64];
                # 3) PE matmul with rep16 replicates to 128 partitions;
                # 4) cast PSUM f32 -> int16.
                idxf = smpool.tile([P, K], F32, tag="idxf")
                nc.vector.tensor_copy(out=idxf[:], in_=idx[:])
                w16f = smpool.tile([16, 64], F32, tag="w16f")
                w16v = w16f[:].rearrange("q (j a) -> q j a", a=8)
                for a in range(8):
                    nc.sync.dma_start(
                        out=w16v[:, :, a], in_=idxf[16 * a : 16 * (a + 1), :]
                    )
                return r8, w16f

            def phase_a2(i, w16f):
                psw = popool.tile([P, 64], F32, tag="po")
                nc.tensor.matmul(
                    out=psw[:], lhsT=rep_sb[:], rhs=w16f[:], start=True, stop=True
                )
                wrapped = smpool.tile([P, 64], I16, tag="wrapped")
                nc.scalar.activation(out=wrapped[:], in_=psw[:], func=AF.Copy)
                g = gpool.tile([P, K * W], BF16)
                gv = g[:].rearrange("p (k w) -> p k w", w=W)
                for part in range(gather_split):
                    kk = K // gather_split
                    nc.gpsimd.dma_gather(
                        out_ap=gv[:, part * kk : (part + 1) * kk, :],
                        in_ap=vals.ap(),
                        idxs_ap=wrapped[:, part * 8 * kk : (part + 1) * 8 * kk],
                        num_idxs=P * kk,
                        num_idxs_reg=P * kk,
                        elem_size=W,
                        queue_num=part % 4,
                    )
                return g

            def phase_b(i, r8, g):
                g3 = g[:].rearrange("p (k w) -> p k w", w=W)
                d8 = smpool.tile([P, K], F32, tag="d8")
                nc.vector.tensor_sub(d8[:], r8[:], g3[:, :, H : H + 1])
                e8 = smpool.tile([P, K], F32, tag="e8")
                den = smpool.tile([P, 1], F32, tag="den")
                nc.scalar.activation(
                    out=e8[:], in_=d8[:], func=AF.Exp, scale=SCALE, accum_out=den[:]
                )
                winv = smpool.tile([P, 1], F32, tag="winv")
                nc.vector.reciprocal(out=winv[:], in_=den[:])

                w8t = smpool.tile([P, K], F32, tag="w8t")
                nc.vector.tensor_scalar(
                    out=w8t[:], in0=e8[:], scalar1=winv[:], scalar2=None,
                    op0=ALU.mult,
                )

                po = popool.tile([P, H], F32, tag="po")
                for k in range(K):
                    dg = dpool.tile([P, P], BF16, tag="dg")
                    nc.scalar.activation(
                        out=dg[:], in_=ident[:], func=AF.Copy,
                        scale=e8[:, k : k + 1],
                    )
                    if mac_wide:
                        nc.tensor.matmul(
                            out=po[:],
                            lhsT=dg[:],
                            rhs=g[:, k * W : k * W + H],
                            start=(k == 0),
                            stop=(k == K - 1),
                        )
                    else:
                        for h2 in range(2):
                            nc.tensor.matmul(
                                out=po[:, h2 * 512 : (h2 + 1) * 512],
                                lhsT=dg[:],
                                rhs=g[:, k * W + h2 * 512 : k * W + (h2 + 1) * 512],
                                start=(k == 0),
                                stop=(k == K - 1),
                            )
                osb = opool.tile([P, H], F32)
                nc.scalar.activation(
                    out=osb[:], in_=po[:], func=AF.Copy, scale=winv[:]
                )
                nc.sync.dma_start(
                    out=out.ap()[i * P : (i + 1) * P, :H], in_=osb[:]
                )
                nc.sync.dma_start(
                    out=out.ap()[i * P : (i + 1) * P, H:], in_=w8t[:]
                )

            # 3-stage software pipeline:
            #  a1(i): scores -> top-k -> shift DMAs (wrap build in flight)
            #  a2(i): wrap matmul + gather launch (deps ready one tile later)
            #  b(i):  post-gather math + MAC + output
            SKEW_A2 = skew_a2
            SKEW_B = skew_b
            stage1 = []  # (i, r8, w16f) awaiting a2
            stage2 = []  # (i, r8, g) awaiting b
            for i in range(n_tiles):
                r8, w16f = phase_a(i)
                stage1.append((i, r8, w16f))
                if len(stage1) > SKEW_A2 - 1:
                    j, r8j, w16fj = stage1.pop(0)
                    gj = phase_a2(j, w16fj)
                    stage2.append((j, r8j, gj))
                if len(stage2) > SKEW_B:
                    j, r8j, gj = stage2.pop(0)
                    phase_b(j, r8j, gj)
            for j, r8j, w16fj in stage1:
                gj = phase_a2(j, w16fj)
                stage2.append((j, r8j, gj))
            for j, r8j, gj in stage2:
                phase_b(j, r8j, gj)
    nc.compile()
    return nc


def make_in_maps(query, keys, values, reliability, tpc=TPC, n_cores=N_CORES):
    query = np.asarray(query, dtype=np.float32)
    keys = np.asarray(keys, dtype=np.float32)
    values = np.asarray(values, dtype=np.float32)
    reliability = np.asarray(reliability, dtype=np.float32)
    qf = query.reshape(-1, D)
    keysT = np.ascontiguousarray(keys.T)
    bias_f = np.log(reliability.reshape(N) + EPS).astype(np.float32)
    vals16 = np.zeros((N, W), dtype=ml_dtypes.bfloat16)
    vals16[:, :H] = values.astype(ml_dtypes.bfloat16)
    vals16[:, H] = bias_f.astype(ml_dtypes.bfloat16)
    bias_row = np.ascontiguousarray(bias_f.reshape(1, N))
    rel2 = reliability.reshape(1, N)
    rep16 = np.zeros((16, P), dtype=np.float32)
    for q in range(16):
        rep16[q, q::16] = 1.0
    in_maps = []
    for c in range(n_cores):
        shard = qf[c * tpc : (c + 1) * tpc]
        in_maps.append(
            {
                "qT": np.ascontiguousarray(shard.T),
                "keysT": keysT,
                "vals": vals16,
                "rel": rel2,
                "bias_in": bias_row,
                "rep16": rep16,
            }
        )
    return in_maps


_CACHED_NC = None


def _get_nc(**bkw):
    global _CACHED_NC
    if _CACHED_NC is None:
        _CACHED_NC = build(**bkw)
    return _CACHED_NC


def run(query, keys, values, reliability, trace=False, build_kwargs=None, **run_kwargs):
    nc = _get_nc(**(build_kwargs or {}))
    in_maps = make_in_maps(query, keys, values, reliability)
    res = run_bass_kernel_spmd(
        nc, in_maps, core_ids=list(range(N_CORES)), trace=trace, **run_kwargs
    )
    full = np.concatenate([res.results[c]["out"] for c in range(N_CORES)], axis=0)
    output = np.ascontiguousarray(full[:, :H]).reshape(B, S, H)
    attn = np.ascontiguousarray(full[:, H:]).reshape(B, S, K)
    return (output, attn), res


def kernel(query, keys, values, reliability):
    (output, attn), _ = run(query, keys, values, reliability, trace=False)
    return output, attn


# revision 38
# speedup vs baseline: 1.2554x; 1.2554x over previous
"""AGA flash-attention (routed slot attention) TRN2 kernel.

Data-parallel over tokens: 16384 tokens split across 8 NeuronCores, slot
memory (keys/values/reliability) replicated. Per core, per 128-token tile:
  1. PE: router scores r = qT.T @ keysT accumulated with a K=1 outer-product
     bias matmul (bias = ln(reliability + eps) per slot).
  2. ACT: drain PSUM -> SBUF.
  3. DVE: max8 (top-8 values, descending) + find_index8 (their slot indices).
  4. GPSIMD indirect DMA: gather bias at the 8 indices (per-token) and the
     8 value rows (bf16) per token.
  5. ACT: e8 = exp((r8-b8)*SCALE) with accumulated denominator; DVE recip.
  6. PE: out = sum_k diag(e8_k) @ V_rows_k (psum f32), drained with a
     per-token 1/denom scale on ACT.  attn_weights = e8 * (1/denom).
Output is packed [tokens, 1024+8] (values || weights) and split on host.
"""

import sys

for _p in ("/opt/trn_rl_repo",):
    if _p not in sys.path:
        sys.path.append(_p)

import numpy as np
import ml_dtypes

import concourse.bass as bass
import concourse.bacc as bacc
import concourse.mybir as mybir
from concourse.tile import TileContext
from concourse.bass_utils import run_bass_kernel_spmd
from concourse import masks

F32 = mybir.dt.float32
I16 = mybir.dt.int16
BF16 = mybir.dt.bfloat16
U32 = mybir.dt.uint32
AF = mybir.ActivationFunctionType
ALU = mybir.AluOpType

B, S, D, N, H, K = 4, 4096, 128, 4096, 1024, 8
N_CORES = 8
TOKENS = B * S
TPC = TOKENS // N_CORES  # tokens per core
P = 128
SCALE = 1.0 / float(np.sqrt(D))
EPS = 1e-10
OUTW = H + K
W = 1152  # augmented row: 1024 values + bias + pad (2304 B, %256 for dma_gather)


def build(tpc=TPC, bias_pe=0, mac_wide=False, skew_a2=3, skew_b=3, gather_split=8, indirect=True):
    n_tiles = tpc // P
    nc = bacc.Bacc("TRN2", target_bir_lowering=False, debug=False, num_swdge_queues=4)
    qT = nc.dram_tensor("qT", [D, tpc], F32, kind="ExternalInput")
    keysT = nc.dram_tensor("keysT", [D, N], F32, kind="ExternalInput")
    vals = nc.dram_tensor("vals", [N, W], BF16, kind="ExternalInput")
    rel = nc.dram_tensor("rel", [1, N], F32, kind="ExternalInput")
    out = nc.dram_tensor("out", [tpc, OUTW], F32, kind="ExternalOutput")
    bias_d = nc.dram_tensor("bias_in", [1, N], F32, kind="ExternalInput")
    rep16 = nc.dram_tensor("rep16", [16, P], F32, kind="ExternalInput")

    with TileContext(nc) as tc:
        with (
            tc.tile_pool(name="const", bufs=1) as cpool,
            tc.tile_pool(name="scores", bufs=2) as spool,
            tc.tile_pool(name="gather", bufs=4) as gpool,
            tc.tile_pool(name="outp", bufs=3) as opool,
            tc.tile_pool(name="small", bufs=10) as smpool,
            tc.tile_pool(name="diag", bufs=8) as dpool,
            tc.tile_pool(name="ps_s", bufs=2, space="PSUM") as pspool,
            tc.tile_pool(name="ps_o", bufs=2, space="PSUM") as popool,
        ):
            qT_sb = cpool.tile([D, tpc], F32)
            nc.sync.dma_start(out=qT_sb[:], in_=qT.ap())
            keysT_sb = cpool.tile([D, N], F32)
            nc.sync.dma_start(out=keysT_sb[:], in_=keysT.ap())
            rel_sb = cpool.tile([1, N], F32)
            nc.sync.dma_start(out=rel_sb[:], in_=rel.ap())
            bias_sb = cpool.tile([1, N], F32)
            nc.sync.dma_start(out=bias_sb[:], in_=bias_d.ap())
            ones_sb = cpool.tile([1, P], F32)
            nc.vector.memset(ones_sb[:], 1.0)
            ident = cpool.tile([P, P], BF16)
            masks.make_identity(nc, ident[:])
            rep_sb = cpool.tile([16, P], F32)
            nc.sync.dma_start(out=rep_sb[:], in_=rep16.ap())
            bias_rep = cpool.tile([P, N], F32)
            nc.gpsimd.partition_broadcast(
                out_ap=bias_rep[:], in_ap=bias_sb[:], channels=P
            )

            def phase_a(i):
                ssb = spool.tile([P, N], F32)
                # chunks 0,1: bias via PE outer-product + ACT drain
                # chunks 2,3: pure score matmul; bias added by the DVE drain
                # (balances PE vs DVE; single qT LDWEIGHTS for all 8 matmuls)
                cand = smpool.tile([P, 32], F32, tag="cand")
                # process chunks in pairs sized to the PSUM pool (2 slots):
                # scores (one qT LDW per pair), then bias matmuls (PE) or
                # bias-added DVE drains, then ACT drains + per-chunk max8.
                for pair in range(2):
                    chunks = (2 * pair, 2 * pair + 1)
                    pss = {}
                    for c in chunks:
                        ps = pspool.tile([P, 1024], F32)
                        pss[c] = ps
                        last = c >= bias_pe
                        for h2 in range(2):
                            sl = slice(c * 1024 + h2 * 512, c * 1024 + (h2 + 1) * 512)
                            nc.tensor.matmul(
                                out=ps[:, h2 * 512 : (h2 + 1) * 512],
                                lhsT=qT_sb[:, i * P : (i + 1) * P],
                                rhs=keysT_sb[:, sl],
                                start=True,
                                stop=last,
                            )
                    for c in chunks:
                        if c < bias_pe:
                            ps = pss[c]
                            for h2 in range(2):
                                sl = slice(
                                    c * 1024 + h2 * 512, c * 1024 + (h2 + 1) * 512
                                )
                                nc.tensor.matmul(
                                    out=ps[:, h2 * 512 : (h2 + 1) * 512],
                                    lhsT=ones_sb[:],
                                    rhs=bias_sb[:, sl],
                                    start=False,
                                    stop=True,
                                )
                    for c in chunks:
                        ps = pss[c]
                        if c < bias_pe:
                            nc.scalar.activation(
                                out=ssb[:, c * 1024 : (c + 1) * 1024],
                                in_=ps[:],
                                func=AF.Copy,
                            )
                        else:
                            nc.vector.scalar_tensor_tensor(
                                out=ssb[:, c * 1024 : (c + 1) * 1024],
                                in0=ps[:],
                                scalar=1.0,
                                in1=bias_rep[:, c * 1024 : (c + 1) * 1024],
                                op0=ALU.mult,
                                op1=ALU.add,
                            )
                        nc.vector.max(
                            out=cand[:, c * 8 : (c + 1) * 8],
                            in_=ssb[:, c * 1024 : (c + 1) * 1024],
                        )

                r8 = smpool.tile([P, K], F32, tag="r8")
                idx = smpool.tile([P, K], U32, tag="idx")
                nc.vector.max(out=r8[:], in_=cand[:])
                nc.vector.max_index(out=idx[:], in_max=r8[:], in_values=ssb[:])

                if indirect:
                    return r8, idx
                # build dma_gather's wrapped int16 index layout on-chip:
                # wrapped[16g+q, 8j+a] = idx[16a+q, j] for all groups g.
                # 1) cast to f32; 2) 8 partition-shift DMAs build [16, 64];
                # 3) PE matmul with rep16 replicates to 128 partitions;
                # 4) cast PSUM f32 -> int16.
                idxf = smpool.tile([P, K], F32, tag="idxf")
                nc.vector.tensor_copy(out=idxf[:], in_=idx[:])
                w16f = smpool.tile([16, 64], F32, tag="w16f")
                w16v = w16f[:].rearrange("q (j a) -> q j a", a=8)
                for a in range(8):
                    nc.sync.dma_start(
                        out=w16v[:, :, a], in_=idxf[16 * a : 16 * (a + 1), :]
                    )
                return r8, w16f

            def phase_a2(i, w16f):
                g = gpool.tile([P, K * W], BF16)
                gv = g[:].rearrange("p (k w) -> p k w", w=W)
                if indirect:
                    idx = w16f
                    for k in range(K):
                        nc.gpsimd.indirect_dma_start(
                            out=g[:, k * W : k * W + 1025],
                            out_offset=None,
                            in_=vals.ap(),
                            in_offset=bass.IndirectOffsetOnAxis(
                                ap=idx[:, k : k + 1], axis=0
                            ),
                        )
                    return g
                psw = popool.tile([P, 64], F32, tag="po")
                nc.tensor.matmul(
                    out=psw[:], lhsT=rep_sb[:], rhs=w16f[:], start=True, stop=True
                )
                wrapped = smpool.tile([P, 64], I16, tag="wrapped")
                nc.scalar.activation(out=wrapped[:], in_=psw[:], func=AF.Copy)
                for part in range(gather_split):
                    kk = K // gather_split
                    nc.gpsimd.dma_gather(
                        out_ap=gv[:, part * kk : (part + 1) * kk, :],
                        in_ap=vals.ap(),
                        idxs_ap=wrapped[:, part * 8 * kk : (part + 1) * 8 * kk],
                        num_idxs=P * kk,
                        num_idxs_reg=P * kk,
                        elem_size=W,
                        queue_num=part % 4,
                    )
                return g

            def phase_b(i, r8, g):
                g3 = g[:].rearrange("p (k w) -> p k w", w=W)
                d8 = smpool.tile([P, K], F32, tag="d8")
                nc.vector.tensor_sub(d8[:], r8[:], g3[:, :, H : H + 1])
                e8 = smpool.tile([P, K], F32, tag="e8")
                den = smpool.tile([P, 1], F32, tag="den")
                nc.scalar.activation(
                    out=e8[:], in_=d8[:], func=AF.Exp, scale=SCALE, accum_out=den[:]
                )
                winv = smpool.tile([P, 1], F32, tag="winv")
                nc.vector.reciprocal(out=winv[:], in_=den[:])

                w8t = smpool.tile([P, K], F32, tag="w8t")
                nc.vector.tensor_scalar(
                    out=w8t[:], in0=e8[:], scalar1=winv[:], scalar2=None,
                    op0=ALU.mult,
                )

                po = popool.tile([P, H], F32, tag="po")
                for k in range(K):
                    dg = dpool.tile([P, P], BF16, tag="dg")
                    nc.scalar.activation(
                        out=dg[:], in_=ident[:], func=AF.Copy,
                        scale=e8[:, k : k + 1],
                    )
                    if mac_wide:
                        nc.tensor.matmul(
                            out=po[:],
                            lhsT=dg[:],
                            rhs=g[:, k * W : k * W + H],
                            start=(k == 0),
                            stop=(k == K - 1),
                        )
                    else:
                        for h2 in range(2):
                            nc.tensor.matmul(
                                out=po[:, h2 * 512 : (h2 + 1) * 512],
                                lhsT=dg[:],
                                rhs=g[:, k * W + h2 * 512 : k * W + (h2 + 1) * 512],
                                start=(k == 0),
                                stop=(k == K - 1),
                            )
                osb = opool.tile([P, H], F32)
                nc.scalar.activation(
                    out=osb[:], in_=po[:], func=AF.Copy, scale=winv[:]
                )
                nc.sync.dma_start(
                    out=out.ap()[i * P : (i + 1) * P, :H], in_=osb[:]
                )
                nc.sync.dma_start(
                    out=out.ap()[i * P : (i + 1) * P, H:], in_=w8t[:]
                )

            # 3-stage software pipeline:
            #  a1(i): scores -> top-k -> shift DMAs (wrap build in flight)
            #  a2(i): wrap matmul + gather launch (deps ready one tile later)
            #  b(i):  post-gather math + MAC + output
            SKEW_A2 = skew_a2
            SKEW_B = skew_b
            stage1 = []  # (i, r8, w16f) awaiting a2
            stage2 = []  # (i, r8, g) awaiting b
            for i in range(n_tiles):
                r8, w16f = phase_a(i)
                stage1.append((i, r8, w16f))
                if len(stage1) > SKEW_A2 - 1:
                    j, r8j, w16fj = stage1.pop(0)
                    gj = phase_a2(j, w16fj)
                    stage2.append((j, r8j, gj))
                if len(stage2) > SKEW_B:
                    j, r8j, gj = stage2.pop(0)
                    phase_b(j, r8j, gj)
            for j, r8j, w16fj in stage1:
                gj = phase_a2(j, w16fj)
                stage2.append((j, r8j, gj))
            for j, r8j, gj in stage2:
                phase_b(j, r8j, gj)
    nc.compile()
    return nc


def make_in_maps(query, keys, values, reliability, tpc=TPC, n_cores=N_CORES):
    query = np.asarray(query, dtype=np.float32)
    keys = np.asarray(keys, dtype=np.float32)
    values = np.asarray(values, dtype=np.float32)
    reliability = np.asarray(reliability, dtype=np.float32)
    qf = query.reshape(-1, D)
    keysT = np.ascontiguousarray(keys.T)
    bias_f = np.log(reliability.reshape(N) + EPS).astype(np.float32)
    vals16 = np.zeros((N, W), dtype=ml_dtypes.bfloat16)
    vals16[:, :H] = values.astype(ml_dtypes.bfloat16)
    vals16[:, H] = bias_f.astype(ml_dtypes.bfloat16)
    bias_row = np.ascontiguousarray(bias_f.reshape(1, N))
    rel2 = reliability.reshape(1, N)
    rep16 = np.zeros((16, P), dtype=np.float32)
    for q in range(16):
        rep16[q, q::16] = 1.0
    in_maps = []
    for c in range(n_cores):
        shard = qf[c * tpc : (c + 1) * tpc]
        in_maps.append(
            {
                "qT": np.ascontiguousarray(shard.T),
                "keysT": keysT,
                "vals": vals16,
                "rel": rel2,
                "bias_in": bias_row,
                "rep16": rep16,
            }
        )
    return in_maps


_CACHED_NC = None


def _get_nc(**bkw):
    global _CACHED_NC
    if _CACHED_NC is None:
        _CACHED_NC = build(**bkw)
    return _CACHED_NC


def run(query, keys, values, reliability, trace=False, build_kwargs=None, **run_kwargs):
    nc = _get_nc(**(build_kwargs or {}))
    in_maps = make_in_maps(query, keys, values, reliability)
    res = run_bass_kernel_spmd(
        nc, in_maps, core_ids=list(range(N_CORES)), trace=trace, **run_kwargs
    )
    full = np.concatenate([res.results[c]["out"] for c in range(N_CORES)], axis=0)
    output = np.ascontiguousarray(full[:, :H]).reshape(B, S, H)
    attn = np.ascontiguousarray(full[:, H:]).reshape(B, S, K)
    return (output, attn), res


def kernel(query, keys, values, reliability):
    (output, attn), _ = run(query, keys, values, reliability, trace=False)
    return output, attn
